# revision 11
# baseline (speedup 1.0000x reference)
"""Trainium2 Bass kernel for nn_Decoder: seq2seq LSTM decoder with attention.

Strategy: 8-way tensor parallel over channels (channels-on-partitions layout).
  - per-core gate-channel slice GCH=512 (128 h-channels x 4 gates)
  - attention scores e: contraction over the core's 128 h-channels -> AllReduce
  - context a: core computes its 256-wide slice of 2H (local, exact)
  - z = U @ W_comb.T: contraction-split over U channels -> partial z -> AllReduce
  - output written pre-tanh in [t, p, (kk b)] device layout; host applies
    tanh + transpose (exact fp32).
Precompute on device: enc_proj (split by (b,s), redistributed via AllToAll) and
pre_t = y_t @ W_ih[:, :E].T + hh_term for all steps.
Host does: embedding gather, hh_term, transposes/casts/slicing of inputs only.
"""

import numpy as np
import ml_dtypes

V, E, H, B, S, T = 32000, 512, 1024, 64, 128, 65
NT = T - 1          # 64 decode steps
R = 8               # cores
HK = H // R         # 128  h-channels per core
GCH = 4 * HK        # 512  gate channels per core
DK = 2 * H // R     # 256  context channels per core
UK = HK + DK        # 384  U channels per core
ESHIFT = 40.0       # softmax stability shift (|e| < 40 measured; exp arg < ~50)

# --- per-site matmul operand dtypes ("f32", "f32r", "bf16") ---
DT_E = "f32"    # e-path: W_att/enc (P1), enc_proj lhsT, h rhs  (chaos-critical)
DT_A = "f32"    # a-path: enc_hidden lhsT, alpha rhs
DT_G = "f32"    # gates: W_ih_o lhsT, o_prev rhs
DT_Z = "f32"    # z: W_comb lhsT, U rhs
DT_P = "f32"    # precompute pre: W_ih_y, Y

_CACHE = {}


def _mybir_dt(name):
    from concourse import mybir
    return {"f32": mybir.dt.float32,
            "f32r": mybir.dt.float32r,
            "bf16": mybir.dt.bfloat16}[name]


def _np_cast(x, name):
    if name == "bf16":
        return np.ascontiguousarray(x).astype(ml_dtypes.bfloat16)
    return np.ascontiguousarray(x).astype(np.float32)


def _build():
    import concourse.bass as bass
    import concourse.mybir as mybir
    import concourse.tile as tile
    from concourse import bacc

    f32 = mybir.dt.float32
    dE, dA, dG, dZ, dP = (_mybir_dt(n) for n in (DT_E, DT_A, DT_G, DT_Z, DT_P))
    AF = mybir.ActivationFunctionType

    nc = bacc.Bacc("TRN2", target_bir_lowering=False, debug=False,
                   enable_asserts=False, num_devices=R)

    # ---- I/O ----
    watt_t = nc.dram_tensor("watt_t", [2 * H, H], dE, kind="ExternalInput")
    enc_bsk = nc.dram_tensor("enc_bsk", [2 * H, B * S // R], dE, kind="ExternalInput")
    enc_sbd = nc.dram_tensor("enc_sbd", [S, B, DK], dA, kind="ExternalInput")
    wihy_t = nc.dram_tensor("wihy_t", [E, GCH], dP, kind="ExternalInput")
    y_t = nc.dram_tensor("y_t", [E, NT * B], dP, kind="ExternalInput")
    hh_in = nc.dram_tensor("hh", [4, HK, B], f32, kind="ExternalInput")
    wiho_t = nc.dram_tensor("wiho_t", [H, GCH], dG, kind="ExternalInput")
    wcomb_t = nc.dram_tensor("wcomb_t", [UK, H], dZ, kind="ExternalInput")
    c0t_in = nc.dram_tensor("c0t", [HK, B], f32, kind="ExternalInput")
    maskb_in = nc.dram_tensor("maskb", [S, B], f32, kind="ExternalInput")
    zout = nc.dram_tensor("zout", [NT, 128, 8 * B], f32, kind="ExternalOutput")

    rg = [list(range(R))]

    with tile.TileContext(nc) as tc:
        with tc.tile_pool(name="res", bufs=1) as res, \
             tc.tile_pool(name="dram", bufs=1, space="DRAM") as dram, \
             tc.tile_pool(name="dbounce", bufs=2, space="DRAM") as dbounce:

            # ---------- resident loads ----------
            wiho_sb = res.tile([128, 8, GCH], dG, name="wiho_sb")
            nc.sync.dma_start(wiho_sb, wiho_t.ap().rearrange("(kk p) c -> p kk c", p=128))
            wcomb_sb = res.tile([128, 3, H], dZ, name="wcomb_sb")
            nc.sync.dma_start(wcomb_sb, wcomb_t.ap().rearrange("(kk p) c -> p kk c", p=128))
            c0_sb = res.tile([128, B], f32, name="c0_sb")
            nc.sync.dma_start(c0_sb, c0t_in.ap())
            maskb_sb = res.tile([128, B], f32, name="maskb_sb")
            nc.sync.dma_start(maskb_sb, maskb_in.ap())
            hh_sb = res.tile([128, 4, B], f32, name="hh_sb")
            nc.sync.dma_start(hh_sb, hh_in.ap().rearrange("m p b -> p m b"))
            ones_col = res.tile([128, 1], f32, name="ones_col")
            nc.gpsimd.memset(ones_col, 1.0)
            ones_row = res.tile([1, 128], f32, name="ones_row")
            nc.gpsimd.memset(ones_row, 1.0)
            encproj_sb = res.tile([128, 8, B * S // R], dE, name="encproj_sb")
            pre_dram = dram.tile([4, NT, 128, B], f32, name="pre_dram")

            # ---------- P1: enc_proj for local (b,s) block, then AllToAll ----------
            with tc.tile_pool(name="p1sb", bufs=2) as p1sb, \
                 tc.tile_pool(name="p1ps", bufs=2, space="PSUM") as p1ps:
                p1in = dram.tile([H, B * S // R], dE, name="p1in")
                p1out = dram.tile([H, B * S // R], dE, name="p1out")
                for ch in range(2):
                    encbsk_sb = p1sb.tile([128, 16, 512], dE, tag="encbsk", bufs=1)
                    nc.sync.dma_start(
                        encbsk_sb,
                        enc_bsk.ap()[:, ch * 512:(ch + 1) * 512]
                        .rearrange("(kk p) c -> p kk c", p=128))
                    for m in range(8):
                        watt_sb = p1sb.tile([128, 16, 128], dE, tag="watt")
                        nc.sync.dma_start(
                            watt_sb,
                            watt_t.ap()[:, m * 128:(m + 1) * 128]
                            .rearrange("(kk p) c -> p kk c", p=128))
                        pt = p1ps.tile([128, 512], f32, tag="p1ps")
                        for kk in range(16):
                            nc.tensor.matmul(
                                pt,
                                watt_sb[:, kk, :],
                                encbsk_sb[:, kk, :],
                                start=(kk == 0), stop=(kk == 15))
                        st = p1sb.tile([128, 512], dE, tag="p1st")
                        nc.any.tensor_copy(st, pt)
                        nc.sync.dma_start(
                            p1in[m * 128:(m + 1) * 128, ch * 512:(ch + 1) * 512], st)
                nc.gpsimd.collective_compute(
                    "AllToAll", mybir.AluOpType.bypass, replica_groups=rg,
                    ins=[p1in.opt()], outs=[p1out.opt()])
                nc.sync.dma_start(
                    encproj_sb, p1out.rearrange("(kk p) c -> p kk c", p=128))

            # ---------- P2: pre[t] = y_t @ Wihy.T + hh ----------
            with tc.tile_pool(name="p2sb", bufs=2) as p2sb, \
                 tc.tile_pool(name="p2ps", bufs=2, space="PSUM") as p2ps:
                wihy_sb = p2sb.tile([128, 4, GCH], dP, name="wihy_sb", bufs=1)
                nc.sync.dma_start(wihy_sb, wihy_t.ap().rearrange("(kk p) c -> p kk c", p=128))
                for tch in range(8):   # chunks of 8 timesteps (512 cols)
                    y_sb = p2sb.tile([128, 4, 512], dP, tag="ych")
                    nc.sync.dma_start(
                        y_sb,
                        y_t.ap()[:, tch * 512:(tch + 1) * 512]
                        .rearrange("(kk p) c -> p kk c", p=128))
                    for m in range(4):
                        pt = p2ps.tile([128, 8, B], f32, tag="p2ps")
                        for kk in range(4):
                            nc.tensor.matmul(
                                pt,
                                wihy_sb[:, kk, m * 128:(m + 1) * 128],
                                y_sb[:, kk, :],
                                start=(kk == 0), stop=(kk == 3))
                        st = p2sb.tile([128, 8, B], f32, tag="p2st")
                        nc.vector.tensor_add(
                            out=st, in0=pt,
                            in1=hh_sb[:, m:m + 1, :].to_broadcast([128, 8, B]))
                        nc.sync.dma_start(
                            pre_dram[m, tch * 8:(tch + 1) * 8].rearrange("t p b -> p t b"),
                            st)

            # ---------- decode loop ----------
            with tc.tile_pool(name="apool", bufs=1) as apool, \
                 tc.tile_pool(name="lp", bufs=2) as lp, \
                 tc.tile_pool(name="ps", bufs=1, space="PSUM") as ps, \
                 tc.tile_pool(name="pso", bufs=2, space="PSUM") as pso:

                # a-path encoder slice (uses space freed by P1/P2 pools)
                encsbd_sb = apool.tile([128, B, DK], dA, name="encsbd_sb")
                nc.sync.dma_start(encsbd_sb, enc_sbd.ap())

                outT = lp.tile([128, 8, B], dG, tag="outT")
                nc.gpsimd.memset(outT, 0.0)

                for t in range(NT):
                    pre_t = lp.tile([128, 4, B], f32, tag="pre_t")
                    nc.sync.dma_start(pre_t, pre_dram[:, t].rearrange("m p b -> p m b"))

                    # gates (partial channels, full contraction over o_prev)
                    pg = ps.tile([128, 4, B], f32, tag="pg")
                    for m in range(4):
                        for kk in range(8):
                            nc.tensor.matmul(
                                pg[:, m, :],
                                wiho_sb[:, kk, m * 128:(m + 1) * 128],
                                outT[:, kk, :],
                                start=(kk == 0), stop=(kk == 7))
                    gf = lp.tile([128, 4, B], f32, tag="gf")
                    nc.vector.tensor_add(out=gf, in0=pg, in1=pre_t)

                    # LSTM pointwise via tanh only (sigmoid_and_others table swap
                    # costs ~2.7us/step; sig(x) = (tanh(x/2)+1)/2):
                    #   2c  = tanh(f/2)*c0 + c0 + tanh(i/2)*tanh(g) + tanh(g)
                    #   2h' = tanh(o/2)*tanh(c) + tanh(c); h = h' (0.5s folded into
                    #   W_att and W_comb h-rows on host)
                    si = lp.tile([128, B], f32, tag="si")
                    nc.scalar.activation(si, gf[:, 0, :], AF.Tanh, scale=0.5)
                    tg = lp.tile([128, B], f32, tag="tg")
                    nc.scalar.activation(tg, gf[:, 2, :], AF.Tanh)
                    sf = lp.tile([128, B], f32, tag="sf")
                    nc.scalar.activation(sf, gf[:, 1, :], AF.Tanh, scale=0.5)
                    so = lp.tile([128, B], f32, tag="so")
                    nc.scalar.activation(so, gf[:, 3, :], AF.Tanh, scale=0.5)
                    t1 = lp.tile([128, B], f32, tag="t1")
                    nc.vector.tensor_mul(out=t1, in0=sf, in1=c0_sb)
                    t2 = lp.tile([128, B], f32, tag="t2")
                    nc.vector.tensor_mul(out=t2, in0=si, in1=tg)
                    t3 = lp.tile([128, B], f32, tag="t3")
                    nc.vector.tensor_add(out=t3, in0=t1, in1=t2)
                    t4 = lp.tile([128, B], f32, tag="t4")
                    nc.vector.tensor_add(out=t4, in0=c0_sb, in1=tg)
                    cc = lp.tile([128, B], f32, tag="cc")
                    nc.vector.tensor_add(out=cc, in0=t3, in1=t4)   # = 2c
                    tcc = lp.tile([128, B], f32, tag="tcc")
                    nc.scalar.activation(tcc, cc, AF.Tanh, scale=0.5)   # tanh(c)
                    t5 = lp.tile([128, B], f32, tag="t5")
                    nc.vector.tensor_mul(out=t5, in0=so, in1=tcc)
                    h_e = lp.tile([128, B], dE, tag="h_e")
                    nc.vector.tensor_add(out=h_e, in0=t5, in1=tcc)  # = 2h
                    u = lp.tile([128, 3, B], dZ, tag="u")
                    nc.any.tensor_copy(u[:, 0, :], h_e)

                    # e partial scores: [s, b] from local h channels
                    pe = ps.tile([128, B], f32, tag="pe")
                    for b in range(B):
                        nc.tensor.matmul(
                            pe[:, b:b + 1],
                            encproj_sb[:, b // 8, (b % 8) * S:(b % 8 + 1) * S],
                            h_e[:, b:b + 1],
                            start=True, stop=True)
                    e_sb = lp.tile([128, B], f32, tag="e_sb")
                    nc.any.tensor_copy(e_sb, pe)
                    ein = dbounce.tile([128, B], f32, tag="ein")
                    eout = dbounce.tile([128, B], f32, tag="eout")
                    nc.sync.dma_start(ein, e_sb)
                    nc.gpsimd.collective_compute(
                        "AllReduce", mybir.AluOpType.add, replica_groups=rg,
                        ins=[ein.opt()], outs=[eout.opt()])
                    ef = lp.tile([128, B], f32, tag="ef")
                    nc.sync.dma_start(ef, eout)

                    # softmax over s (partition dim) without max-subtraction
                    eb = lp.tile([128, B], f32, tag="eb")
                    nc.vector.tensor_add(out=eb, in0=ef, in1=maskb_sb)
                    ex = lp.tile([128, B], f32, tag="ex")
                    nc.scalar.activation(ex, eb, AF.Exp)
                    pd = ps.tile([1, B], f32, tag="pd")
                    nc.tensor.matmul(pd, ones_col, ex, start=True, stop=True)
                    rd = lp.tile([1, B], f32, tag="rd")
                    nc.vector.reciprocal(rd, pd)
                    pr = ps.tile([128, B], f32, tag="pr")
                    nc.tensor.matmul(pr, ones_row, rd, start=True, stop=True)
                    al = lp.tile([128, B], dA, tag="al")
                    nc.vector.tensor_mul(out=al, in0=ex, in1=pr)

                    # a slice: [d_k, b] (exact, local)
                    pa = ps.tile([128, 2, B], f32, tag="pa")
                    for b in range(B):
                        for dti in range(2):
                            nc.tensor.matmul(
                                pa[:, dti, b:b + 1],
                                encsbd_sb[:, b, dti * 128:(dti + 1) * 128],
                                al[:, b:b + 1],
                                start=True, stop=True)
                    nc.any.tensor_copy(u[:, 1, :], pa[:, 0, :])
                    nc.any.tensor_copy(u[:, 2, :], pa[:, 1, :])

                    # z partial: [H, b] over local U channels
                    pz = pso.tile([128, 8, B], f32, tag="pz")
                    for m in range(8):
                        for kk in range(3):
                            nc.tensor.matmul(
                                pz[:, m, :],
                                wcomb_sb[:, kk, m * 128:(m + 1) * 128],
                                u[:, kk, :],
                                start=(kk == 0), stop=(kk == 2))
                    zs = lp.tile([128, 8, B], f32, tag="zs")
                    nc.any.tensor_copy(zs, pz)
                    zin = dbounce.tile([128, 8 * B], f32, tag="zin")
                    zout_b = dbounce.tile([128, 8 * B], f32, tag="zout_b")
                    nc.sync.dma_start(zin, zs)
                    nc.gpsimd.collective_compute(
                        "AllReduce", mybir.AluOpType.add, replica_groups=rg,
                        ins=[zin.opt()], outs=[zout_b.opt()])
                    zf = lp.tile([128, 8, B], f32, tag="zf")
                    nc.sync.dma_start(zf, zout_b)

                    # output (pre-tanh; host finishes) + recurrent OUT.T
                    nc.sync.dma_start(zout.ap()[t], zf)
                    outT = lp.tile([128, 8, B], dG, tag="outT")
                    nc.scalar.activation(outT, zf, AF.Tanh)

    nc.compile()
    return nc


def _get_nc():
    if "nc" not in _CACHE:
        _CACHE["nc"] = _build()
    return _CACHE["nc"]


def kernel(**inputs):
    tok = np.asarray(inputs["tgt_token_ids"])[:NT]
    enc = np.asarray(inputs["enc_hidden"], dtype=np.float32)
    mask = np.asarray(inputs["enc_mask"])
    h0 = np.asarray(inputs["dec_init_hidden"], dtype=np.float32)
    c0 = np.asarray(inputs["dec_init_cell"], dtype=np.float32)
    emb = np.asarray(inputs["embedding"], dtype=np.float32)
    Wih = np.asarray(inputs["W_ih"], dtype=np.float32)
    Whh = np.asarray(inputs["W_hh"], dtype=np.float32)
    bih = np.asarray(inputs["b_ih"], dtype=np.float32)
    bhh = np.asarray(inputs["b_hh"], dtype=np.float32)
    Watt = np.asarray(inputs["W_att"], dtype=np.float32)
    Wc = np.asarray(inputs["W_comb"], dtype=np.float32)

    Y = emb[tok]                                   # [NT, B, E]
    hh = h0 @ Whh.T + bih + bhh                    # [B, 4H]
    encT = np.ascontiguousarray(enc.transpose(2, 0, 1)).reshape(2 * H, B * S)
    encSBD = enc.transpose(1, 0, 2)                # [S, B, 2H]
    maskb = (np.where(mask.astype(bool), -1e30, 0.0).T - ESHIFT).astype(np.float32)
    YT = np.ascontiguousarray(Y.reshape(NT * B, E).T)   # [E, NT*B]

    in_maps = []
    for k in range(R):
        gch = np.concatenate(
            [np.arange(g * H + k * HK, g * H + (k + 1) * HK) for g in range(4)])
        uidx = np.r_[k * HK:(k + 1) * HK, H + k * DK:H + (k + 1) * DK]
        # h is produced as 2h on device; fold the 0.5 into the two consumers
        wcomb_k = Wc[:, uidx].T.copy()
        wcomb_k[:HK] *= 0.5
        in_maps.append({
            "watt_t": _np_cast(0.5 * Watt.T, DT_E),
            "enc_bsk": _np_cast(encT[:, k * (B * S // R):(k + 1) * (B * S // R)], DT_E),
            "enc_sbd": _np_cast(encSBD[:, :, k * DK:(k + 1) * DK], DT_A),
            "wihy_t": _np_cast(Wih[gch, :E].T, DT_P),
            "y_t": _np_cast(YT, DT_P),
            "hh": np.ascontiguousarray(hh[:, gch].T.reshape(4, HK, B)).astype(np.float32),
            "wiho_t": _np_cast(Wih[gch, E:].T, DT_G),
            "wcomb_t": _np_cast(wcomb_k, DT_Z),
            "c0t": np.ascontiguousarray(c0[:, k * HK:(k + 1) * HK].T).astype(np.float32),
            "maskb": np.ascontiguousarray(maskb),
        })

    from concourse.bass_utils import run_bass_kernel_spmd
    nc = _get_nc()
    globals()["LAST_IN_MAPS"] = in_maps
    res = run_bass_kernel_spmd(nc, in_maps, core_ids=list(range(R)))
    globals()["LAST_RESULTS"] = res
    z = res.results[0]["zout"].reshape(NT, 128, 8, B)
    out = np.tanh(z.transpose(0, 3, 2, 1).reshape(NT, B, H)).astype(np.float32)
    return out


if __name__ == "__main__":
    nc = _build()
    print("build OK")


# revision 17
# speedup vs baseline: 1.1450x; 1.1450x over previous
"""Trainium2 Bass kernel for nn_Decoder: seq2seq LSTM decoder with attention.

Strategy: 8-way tensor parallel over channels (channels-on-partitions layout).
  - per-core gate-channel slice GCH=512 (128 h-channels x 4 gates)
  - attention scores e: contraction over the core's 128 h-channels -> AllReduce
  - context a: core computes its 256-wide slice of 2H (local, exact)
  - z = U @ W_comb.T: contraction-split over U channels -> partial z -> AllReduce
  - output written pre-tanh in [t, p, (kk b)] device layout; host applies
    tanh + transpose (exact fp32).
Precompute on device: enc_proj (split by (b,s), redistributed via AllToAll) and
pre_t = y_t @ W_ih[:, :E].T + hh_term for all steps.
Host does: embedding gather, hh_term, transposes/casts/slicing of inputs only.
"""

import numpy as np
import ml_dtypes

V, E, H, B, S, T = 32000, 512, 1024, 64, 128, 65
NT = T - 1          # 64 decode steps
R = 8               # cores
HK = H // R         # 128  h-channels per core
GCH = 4 * HK        # 512  gate channels per core
DK = 2 * H // R     # 256  context channels per core
UK = HK + DK        # 384  U channels per core
ESHIFT = 40.0       # softmax stability shift (|e| < 40 measured; exp arg < ~50)

# --- per-site matmul operand dtypes ("f32", "f32r", "bf16") ---
DT_E = "f32"    # e-path: W_att/enc (P1), enc_proj lhsT, h rhs  (chaos-critical)
DT_A = "f32"    # a-path: enc_hidden lhsT, alpha rhs
DT_G = "f32"    # gates: W_ih_o lhsT, o_prev rhs
DT_Z = "f32"    # z: W_comb lhsT, U rhs
DT_P = "f32"    # precompute pre: W_ih_y, Y

_CACHE = {}
COMM_MODE = __import__("os").environ.get("KERNEL_COMM_MODE", "ar")
LITE = __import__("os").environ.get("KERNEL_LITE", "0") == "1"  # timing probe only


def _mybir_dt(name):
    from concourse import mybir
    return {"f32": mybir.dt.float32,
            "f32r": mybir.dt.float32r,
            "bf16": mybir.dt.bfloat16}[name]


def _np_cast(x, name):
    if name == "bf16":
        return np.ascontiguousarray(x).astype(ml_dtypes.bfloat16)
    return np.ascontiguousarray(x).astype(np.float32)


def _build():
    import concourse.bass as bass
    import concourse.mybir as mybir
    import concourse.tile as tile
    from concourse import bacc

    f32 = mybir.dt.float32
    dE, dA, dG, dZ, dP = (_mybir_dt(n) for n in (DT_E, DT_A, DT_G, DT_Z, DT_P))
    AF = mybir.ActivationFunctionType

    nc = bacc.Bacc("TRN2", target_bir_lowering=False, debug=False,
                   enable_asserts=False, num_devices=R)

    # ---- I/O ----
    watt_t = nc.dram_tensor("watt_t", [2 * H, H], dE, kind="ExternalInput")
    enc_bsk = nc.dram_tensor("enc_bsk", [2 * H, B * S // R], dE, kind="ExternalInput")
    enc_sbd = nc.dram_tensor("enc_sbd", [S, B, DK], dA, kind="ExternalInput")
    wihy_t = nc.dram_tensor("wihy_t", [E, GCH], dP, kind="ExternalInput")
    y_t = nc.dram_tensor("y_t", [E, NT * B], dP, kind="ExternalInput")
    hh_in = nc.dram_tensor("hh", [4, HK, B], f32, kind="ExternalInput")
    wiho_t = nc.dram_tensor("wiho_t", [H, GCH], dG, kind="ExternalInput")
    wcomb_t = nc.dram_tensor("wcomb_t", [UK, H], dZ, kind="ExternalInput")
    c0t_in = nc.dram_tensor("c0t", [HK, B], f32, kind="ExternalInput")
    maskb_in = nc.dram_tensor("maskb", [S, B], f32, kind="ExternalInput")
    zout = nc.dram_tensor("zout", [NT, 128, 8 * B], f32, kind="ExternalOutput")

    rg = [list(range(R))]

    with tile.TileContext(nc) as tc:
        with tc.tile_pool(name="res", bufs=1) as res, \
             tc.tile_pool(name="dram", bufs=1, space="DRAM") as dram, \
             tc.tile_pool(name="dbounce", bufs=2, space="DRAM") as dbounce:

            # ---------- resident loads ----------
            wiho_sb = res.tile([128, 8, GCH], dG, name="wiho_sb")
            nc.sync.dma_start(wiho_sb, wiho_t.ap().rearrange("(kk p) c -> p kk c", p=128))
            wcomb_sb = res.tile([128, 3, H], dZ, name="wcomb_sb")
            nc.sync.dma_start(wcomb_sb, wcomb_t.ap().rearrange("(kk p) c -> p kk c", p=128))
            c0_sb = res.tile([128, B], f32, name="c0_sb")
            nc.sync.dma_start(c0_sb, c0t_in.ap())
            maskb_sb = res.tile([128, B], f32, name="maskb_sb")
            nc.sync.dma_start(maskb_sb, maskb_in.ap())
            hh_sb = res.tile([128, 4, B], f32, name="hh_sb")
            nc.sync.dma_start(hh_sb, hh_in.ap().rearrange("m p b -> p m b"))
            ones_col = res.tile([128, 1], f32, name="ones_col")
            nc.gpsimd.memset(ones_col, 1.0)
            ones_row = res.tile([1, 128], f32, name="ones_row")
            nc.gpsimd.memset(ones_row, 1.0)
            encproj_sb = res.tile([128, 8, B * S // R], dE, name="encproj_sb")
            pre_dram = dram.tile([4, NT, 128, B], f32, name="pre_dram")

            # ---------- P1: enc_proj for local (b,s) block, then AllToAll ----------
            with tc.tile_pool(name="p1sb", bufs=2) as p1sb, \
                 tc.tile_pool(name="p1ps", bufs=2, space="PSUM") as p1ps:
                p1in = dram.tile([H, B * S // R], dE, name="p1in")
                p1out = dram.tile([H, B * S // R], dE, name="p1out")
                for ch in range(2):
                    encbsk_sb = p1sb.tile([128, 16, 512], dE, tag="encbsk", bufs=1)
                    nc.sync.dma_start(
                        encbsk_sb,
                        enc_bsk.ap()[:, ch * 512:(ch + 1) * 512]
                        .rearrange("(kk p) c -> p kk c", p=128))
                    for m in range(8):
                        watt_sb = p1sb.tile([128, 16, 128], dE, tag="watt")
                        nc.sync.dma_start(
                            watt_sb,
                            watt_t.ap()[:, m * 128:(m + 1) * 128]
                            .rearrange("(kk p) c -> p kk c", p=128))
                        pt = p1ps.tile([128, 512], f32, tag="p1ps")
                        for kk in range(16):
                            nc.tensor.matmul(
                                pt,
                                watt_sb[:, kk, :],
                                encbsk_sb[:, kk, :],
                                start=(kk == 0), stop=(kk == 15))
                        st = p1sb.tile([128, 512], dE, tag="p1st")
                        nc.any.tensor_copy(st, pt)
                        nc.sync.dma_start(
                            p1in[m * 128:(m + 1) * 128, ch * 512:(ch + 1) * 512], st)
                nc.gpsimd.collective_compute(
                    "AllToAll", mybir.AluOpType.bypass, replica_groups=rg,
                    ins=[p1in.opt()], outs=[p1out.opt()])
                nc.sync.dma_start(
                    encproj_sb, p1out.rearrange("(kk p) c -> p kk c", p=128))

            # ---------- P2: pre[t] = y_t @ Wihy.T + hh ----------
            with tc.tile_pool(name="p2sb", bufs=2) as p2sb, \
                 tc.tile_pool(name="p2ps", bufs=2, space="PSUM") as p2ps:
                wihy_sb = p2sb.tile([128, 4, GCH], dP, name="wihy_sb", bufs=1)
                nc.sync.dma_start(wihy_sb, wihy_t.ap().rearrange("(kk p) c -> p kk c", p=128))
                for tch in range(8):   # chunks of 8 timesteps (512 cols)
                    y_sb = p2sb.tile([128, 4, 512], dP, tag="ych")
                    nc.sync.dma_start(
                        y_sb,
                        y_t.ap()[:, tch * 512:(tch + 1) * 512]
                        .rearrange("(kk p) c -> p kk c", p=128))
                    for m in range(4):
                        pt = p2ps.tile([128, 8, B], f32, tag="p2ps")
                        for kk in range(4):
                            nc.tensor.matmul(
                                pt,
                                wihy_sb[:, kk, m * 128:(m + 1) * 128],
                                y_sb[:, kk, :],
                                start=(kk == 0), stop=(kk == 3))
                        st = p2sb.tile([128, 8, B], f32, tag="p2st")
                        nc.vector.tensor_add(
                            out=st, in0=pt,
                            in1=hh_sb[:, m:m + 1, :].to_broadcast([128, 8, B]))
                        nc.sync.dma_start(
                            pre_dram[m, tch * 8:(tch + 1) * 8].rearrange("t p b -> p t b"),
                            st)

            # ---------- decode loop ----------
            with tc.tile_pool(name="apool", bufs=1) as apool, \
                 tc.tile_pool(name="lp", bufs=2) as lp, \
                 tc.tile_pool(name="ps", bufs=1, space="PSUM") as ps, \
                 tc.tile_pool(name="pso", bufs=2, space="PSUM") as pso:

                # a-path encoder slice (uses space freed by P1/P2 pools)
                encsbd_sb = apool.tile([128, B, DK], dA, name="encsbd_sb")
                nc.sync.dma_start(encsbd_sb, enc_sbd.ap())

                outT = lp.tile([128, 8, B], dG, tag="outT")
                nc.gpsimd.memset(outT, 0.0)

                for t in range(NT):
                    pre_t = lp.tile([128, 4, B], f32, tag="pre_t")
                    nc.sync.dma_start(pre_t, pre_dram[:, t].rearrange("m p b -> p m b"))

                    # gates (partial channels, full contraction over o_prev)
                    pg = ps.tile([128, 4, B], f32, tag="pg")
                    for m in range(4):
                        for kk in range(8):
                            nc.tensor.matmul(
                                pg[:, m, :],
                                wiho_sb[:, kk, m * 128:(m + 1) * 128],
                                outT[:, kk, :],
                                start=(kk == 0), stop=(kk == 7))
                    gf = lp.tile([128, 4, B], f32, tag="gf")
                    nc.vector.tensor_add(out=gf, in0=pg, in1=pre_t)

                    # LSTM pointwise via tanh only (sigmoid_and_others table swap
                    # costs ~2.7us/step; sig(x) = (tanh(x/2)+1)/2):
                    #   2c  = tanh(f/2)*c0 + c0 + tanh(i/2)*tanh(g) + tanh(g)
                    #   2h' = tanh(o/2)*tanh(c) + tanh(c); h = h' (0.5s folded into
                    #   W_att and W_comb h-rows on host)
                    si = lp.tile([128, B], f32, tag="si")
                    nc.scalar.activation(si, gf[:, 0, :], AF.Tanh, scale=0.5)
                    tg = lp.tile([128, B], f32, tag="tg")
                    nc.scalar.activation(tg, gf[:, 2, :], AF.Tanh)
                    sf = lp.tile([128, B], f32, tag="sf")
                    nc.scalar.activation(sf, gf[:, 1, :], AF.Tanh, scale=0.5)
                    so = lp.tile([128, B], f32, tag="so")
                    nc.scalar.activation(so, gf[:, 3, :], AF.Tanh, scale=0.5)
                    t1 = lp.tile([128, B], f32, tag="t1")
                    nc.vector.tensor_mul(out=t1, in0=sf, in1=c0_sb)
                    t2 = lp.tile([128, B], f32, tag="t2")
                    nc.vector.tensor_mul(out=t2, in0=si, in1=tg)
                    t3 = lp.tile([128, B], f32, tag="t3")
                    nc.vector.tensor_add(out=t3, in0=t1, in1=t2)
                    t4 = lp.tile([128, B], f32, tag="t4")
                    nc.vector.tensor_add(out=t4, in0=c0_sb, in1=tg)
                    cc = lp.tile([128, B], f32, tag="cc")
                    nc.vector.tensor_add(out=cc, in0=t3, in1=t4)   # = 2c
                    tcc = lp.tile([128, B], f32, tag="tcc")
                    nc.scalar.activation(tcc, cc, AF.Tanh, scale=0.5)   # tanh(c)
                    t5 = lp.tile([128, B], f32, tag="t5")
                    nc.vector.tensor_mul(out=t5, in0=so, in1=tcc)
                    h_e = lp.tile([128, B], dE, tag="h_e")
                    nc.vector.tensor_add(out=h_e, in0=t5, in1=tcc)  # = 2h
                    u = lp.tile([128, 3, B], dZ, tag="u")
                    nc.any.tensor_copy(u[:, 0, :], h_e)

                    # e partial scores: [s, b] from local h channels
                    pe = ps.tile([128, B], f32, tag="pe")
                    for b in range(8 if LITE else B):
                        nc.tensor.matmul(
                            pe[:, b:b + 1],
                            encproj_sb[:, b // 8, (b % 8) * S:(b % 8 + 1) * S],
                            h_e[:, b:b + 1],
                            start=True, stop=True)
                    e_sb = lp.tile([128, B], f32, tag="e_sb")
                    nc.any.tensor_copy(e_sb, pe)
                    ein = dbounce.tile([128, B], f32, tag="ein")
                    eout = dbounce.tile([128, B], f32, tag="eout")
                    nc.sync.dma_start(ein, e_sb)
                    if COMM_MODE == "ar":
                        nc.gpsimd.collective_compute(
                            "AllReduce", mybir.AluOpType.add, replica_groups=rg,
                            ins=[ein.opt()], outs=[eout.opt()])
                    else:
                        nc.sync.dma_start(eout, ein)
                    ef = lp.tile([128, B], f32, tag="ef")
                    nc.sync.dma_start(ef, eout)

                    # softmax over s (partition dim) without max-subtraction
                    eb = lp.tile([128, B], f32, tag="eb")
                    nc.vector.tensor_add(out=eb, in0=ef, in1=maskb_sb)
                    ex = lp.tile([128, B], f32, tag="ex")
                    nc.scalar.activation(ex, eb, AF.Exp)
                    pd = ps.tile([1, B], f32, tag="pd")
                    nc.tensor.matmul(pd, ones_col, ex, start=True, stop=True)
                    rd = lp.tile([1, B], f32, tag="rd")
                    nc.vector.reciprocal(rd, pd)
                    pr = ps.tile([128, B], f32, tag="pr")
                    nc.tensor.matmul(pr, ones_row, rd, start=True, stop=True)
                    al = lp.tile([128, B], dA, tag="al")
                    nc.vector.tensor_mul(out=al, in0=ex, in1=pr)

                    # a slice: [d_k, b] (exact, local)
                    pa = ps.tile([128, 2, B], f32, tag="pa")
                    for b in range(8 if LITE else B):
                        for dti in range(2):
                            nc.tensor.matmul(
                                pa[:, dti, b:b + 1],
                                encsbd_sb[:, b, dti * 128:(dti + 1) * 128],
                                al[:, b:b + 1],
                                start=True, stop=True)
                    nc.any.tensor_copy(u[:, 1, :], pa[:, 0, :])
                    nc.any.tensor_copy(u[:, 2, :], pa[:, 1, :])

                    # z partial: [H, b] over local U channels
                    pz = pso.tile([128, 8, B], f32, tag="pz")
                    for m in range(8):
                        for kk in range(3):
                            nc.tensor.matmul(
                                pz[:, m, :],
                                wcomb_sb[:, kk, m * 128:(m + 1) * 128],
                                u[:, kk, :],
                                start=(kk == 0), stop=(kk == 2))
                    zs = lp.tile([128, 8, B], f32, tag="zs")
                    nc.any.tensor_copy(zs, pz)
                    zin = dbounce.tile([128, 8 * B], f32, tag="zin")
                    zout_b = dbounce.tile([128, 8 * B], f32, tag="zout_b")
                    nc.sync.dma_start(zin, zs)
                    if COMM_MODE == "ar":
                        nc.gpsimd.collective_compute(
                            "AllReduce", mybir.AluOpType.add, replica_groups=rg,
                            ins=[zin.opt()], outs=[zout_b.opt()])
                    else:
                        nc.sync.dma_start(zout_b, zin)
                    zf = lp.tile([128, 8, B], f32, tag="zf")
                    nc.sync.dma_start(zf, zout_b)

                    # output (pre-tanh; host finishes) + recurrent OUT.T
                    nc.sync.dma_start(zout.ap()[t], zf)
                    outT = lp.tile([128, 8, B], dG, tag="outT")
                    nc.scalar.activation(outT, zf, AF.Tanh)

    nc.compile()
    return nc


def _get_nc():
    if "nc" not in _CACHE:
        _CACHE["nc"] = _build()
    return _CACHE["nc"]


def kernel(**inputs):
    tok = np.asarray(inputs["tgt_token_ids"])[:NT]
    enc = np.asarray(inputs["enc_hidden"], dtype=np.float32)
    mask = np.asarray(inputs["enc_mask"])
    h0 = np.asarray(inputs["dec_init_hidden"], dtype=np.float32)
    c0 = np.asarray(inputs["dec_init_cell"], dtype=np.float32)
    emb = np.asarray(inputs["embedding"], dtype=np.float32)
    Wih = np.asarray(inputs["W_ih"], dtype=np.float32)
    Whh = np.asarray(inputs["W_hh"], dtype=np.float32)
    bih = np.asarray(inputs["b_ih"], dtype=np.float32)
    bhh = np.asarray(inputs["b_hh"], dtype=np.float32)
    Watt = np.asarray(inputs["W_att"], dtype=np.float32)
    Wc = np.asarray(inputs["W_comb"], dtype=np.float32)

    Y = emb[tok]                                   # [NT, B, E]
    hh = h0 @ Whh.T + bih + bhh                    # [B, 4H]
    encT = np.ascontiguousarray(enc.transpose(2, 0, 1)).reshape(2 * H, B * S)
    encSBD = enc.transpose(1, 0, 2)                # [S, B, 2H]
    maskb = (np.where(mask.astype(bool), -1e30, 0.0).T - ESHIFT).astype(np.float32)
    YT = np.ascontiguousarray(Y.reshape(NT * B, E).T)   # [E, NT*B]

    in_maps = []
    for k in range(R):
        gch = np.concatenate(
            [np.arange(g * H + k * HK, g * H + (k + 1) * HK) for g in range(4)])
        uidx = np.r_[k * HK:(k + 1) * HK, H + k * DK:H + (k + 1) * DK]
        # h is produced as 2h on device; fold the 0.5 into the two consumers
        wcomb_k = Wc[:, uidx].T.copy()
        wcomb_k[:HK] *= 0.5
        in_maps.append({
            "watt_t": _np_cast(0.5 * Watt.T, DT_E),
            "enc_bsk": _np_cast(encT[:, k * (B * S // R):(k + 1) * (B * S // R)], DT_E),
            "enc_sbd": _np_cast(encSBD[:, :, k * DK:(k + 1) * DK], DT_A),
            "wihy_t": _np_cast(Wih[gch, :E].T, DT_P),
            "y_t": _np_cast(YT, DT_P),
            "hh": np.ascontiguousarray(hh[:, gch].T.reshape(4, HK, B)).astype(np.float32),
            "wiho_t": _np_cast(Wih[gch, E:].T, DT_G),
            "wcomb_t": _np_cast(wcomb_k, DT_Z),
            "c0t": np.ascontiguousarray(c0[:, k * HK:(k + 1) * HK].T).astype(np.float32),
            "maskb": np.ascontiguousarray(maskb),
        })

    from concourse.bass_utils import run_bass_kernel_spmd
    nc = _get_nc()
    globals()["LAST_IN_MAPS"] = in_maps
    res = run_bass_kernel_spmd(nc, in_maps, core_ids=list(range(R)))
    globals()["LAST_RESULTS"] = res
    z = res.results[0]["zout"].reshape(NT, 128, 8, B)
    out = np.tanh(z.transpose(0, 3, 2, 1).reshape(NT, B, H)).astype(np.float32)
    return out


if __name__ == "__main__":
    nc = _build()
    print("build OK")
